# revision 1
# baseline (speedup 1.0000x reference)
"""Trainium2 Bass kernel for nn_CLIP topk_masking.

Computes, for full inputs (self-contained; shapes hardcoded):
    probability = image_features @ ima_proto.T          # [B, NP]
    thr_r       = k-th largest of probability row r
    sel[r, j]   = probability[r, j] >= thr_r            # top-k prototype mask
    text_n      = exp(logit_scale) * text_raw / ||text_raw||_row
    logits[r,c] = (image_features @ text_n.T)[r,c] * sel[r, c // 10]

Sharding: data-parallel over the batch axis across 8 NeuronCores
(rows 512/core); prototypes and text features replicated.
"""

import os
from contextlib import ExitStack

import numpy as np

import concourse.bass as bass
import concourse.tile as tile
from concourse import bacc, mybir
from concourse.bass_utils import run_bass_kernel_spmd

# Problem shapes (hardcoded per contract).
B, D, NP, NC, CPT = 4096, 512, 1000, 10000, 10
NCORES = 8
RLOC = B // NCORES          # 512 rows per core
RT = RLOC // 128            # 4 row tiles per core
KD = D // 128               # 4 contraction chunks
CT = 125                    # classes per text/proto tile (1000 = 8*125, 10000 = 80*125)
CHW = 500                   # class chunk width for matmul N (= 50 proto blocks)
TPC = CHW // CT             # 4 text tiles per chunk
NCH = NC // CHW             # 20 chunks
GRP = 4                     # chunks per output stage group (2000 cols per DMA)
PAIR = 2                    # text chunks loaded per DMA (2 MB transfers)
NEG = -1.0e30

F32 = mybir.dt.float32
F32R = mybir.dt.float32r

LAST_RESULTS = None


def _emit(ctx: ExitStack, tc, img, proto, text, out, k: int, inv_s2: float):
    nc = tc.nc
    AF = mybir.ActivationFunctionType
    OP = mybir.AluOpType

    const = ctx.enter_context(tc.tile_pool(name="const", bufs=1))
    persist = ctx.enter_context(tc.tile_pool(name="persist", bufs=1))

    # Identity matrix for PE transposes.
    ones = const.tile([128, 128], F32)
    nc.vector.memset(ones[:], 1.0)
    ident = const.tile([128, 128], F32)
    nc.gpsimd.affine_select(
        ident[:], ones[:], pattern=[[1, 128]], compare_op=OP.is_equal,
        fill=0.0, base=0, channel_multiplier=-1,
    )

    # imgT[p, kc, r] = img[r, kc*128 + p]; sel[rt][p, j] = top-k mask row 128*rt+p.
    imgT = persist.tile([128, KD, RLOC], F32)
    imgT_r = persist.tile([128, KD, RLOC], F32R)
    sels = []

    # Text chunk-pair loads (2 MB each) on the Sync HWDGE queue. The pool is
    # opened before phase A so the first two pairs prefetch during it.
    pb_traw = ctx.enter_context(tc.tile_pool(name="pb_traw", bufs=3))
    traw_tiles = {}

    def load_pair(pair: int):
        t_ = pb_traw.tile([CT, PAIR * TPC, D], F32, name=f"traw{pair}", tag="traw")
        nc.sync.dma_start(
            t_[:], text[pair * PAIR * CHW:(pair + 1) * PAIR * CHW].rearrange(
                "(t p) d -> p t d", p=CT))
        traw_tiles[pair] = t_

    load_pair(0)
    load_pair(1)

    # ---------- Phase A: img/proto transpose, probability matmul, top-k ----------
    with (
        tc.tile_pool(name="pa_sb", bufs=1) as pa_sb,
        tc.tile_pool(name="pa_ps", bufs=2, space="PSUM") as pa_ps,
        tc.tile_pool(name="pa_prob_ps", bufs=2, space="PSUM") as pa_prob_ps,
        tc.tile_pool(name="pa_work", bufs=2) as pa_work,
    ):
        # img/proto loads on the SWDGE (gpsimd) queue so they don't serialize
        # behind text-chunk loads on the Sync HWDGE queue.
        img_sb = pa_sb.tile([128, RT, D], F32)
        nc.gpsimd.dma_start(img_sb[:], img.rearrange("(t p) d -> p t d", p=128))
        for rt in range(RT):
            for kc in range(KD):
                pi = pa_ps.tile([128, 128], F32, tag="pi")
                nc.tensor.transpose(
                    pi[:], img_sb[:, rt, kc * 128:(kc + 1) * 128], ident[:])
                nc.vector.tensor_copy(imgT[:, kc, rt * 128:(rt + 1) * 128], pi[:])
        # tf32-rounded copy for the fp32r logit matmul.
        nc.vector.tensor_copy(imgT_r[:], imgT[:])

        proto_sb = pa_sb.tile([CT, NP // CT, D], F32)
        nc.gpsimd.dma_start(proto_sb[:], proto.rearrange("(t p) d -> p t d", p=CT))
        protoT = pa_sb.tile([128, KD, NP], F32)
        for t in range(NP // CT):
            pp = pa_ps.tile([128, KD, CT], F32, tag="pp")
            for kc in range(KD):
                nc.tensor.transpose(
                    pp[:, kc], proto_sb[:, t, kc * 128:(kc + 1) * 128],
                    ident[:CT, :CT])
            nc.vector.tensor_copy(protoT[:, :, t * CT:(t + 1) * CT], pp[:])

        for rt in range(RT):
            ppr = pa_prob_ps.tile([128, 2, 512], F32)
            for h in range(2):
                for kc in range(KD):
                    # fp32 (not fp32r): ranking precision decides the mask.
                    nc.tensor.matmul(
                        ppr[:, h, :NP // 2],
                        imgT[:, kc, rt * 128:(rt + 1) * 128],
                        protoT[:, kc, h * (NP // 2):(h + 1) * (NP // 2)],
                        start=(kc == 0), stop=(kc == KD - 1),
                    )
            prob = pa_work.tile([128, NP], F32, tag="prob")
            nc.vector.tensor_copy(
                prob[:].rearrange("p (a b) -> p a b", a=2), ppr[:, :, :NP // 2])
            m8a = pa_work.tile([128, 8], F32, tag="m8a")
            nc.vector.max(m8a[:], prob[:])
            if k <= 8:
                thr = m8a[:, k - 1:k]
            else:
                repl = pa_work.tile([128, NP], F32, tag="repl")
                nc.vector.match_replace(repl[:], m8a[:], prob[:], NEG)
                m8b = pa_work.tile([128, 8], F32, tag="m8b")
                nc.vector.max(m8b[:], repl[:])
                thr = m8b[:, k - 9:k - 8]
            sel = persist.tile([128, NP], F32, tag=f"sel{rt}")
            nc.vector.tensor_scalar(sel[:], prob[:], thr, None, op0=OP.is_ge)
            sels.append(sel)

    # ---------- Phase B: text normalize+transpose, logit matmul, mask, store ----------
    with (
        tc.tile_pool(name="pb_nrm", bufs=2) as pb_nrm,
        tc.tile_pool(name="pb_sq", bufs=2) as pb_sq,
        tc.tile_pool(name="pb_sc", bufs=4) as pb_sc,
        tc.tile_pool(name="pb_ttT", bufs=3) as pb_ttT,
        tc.tile_pool(name="pb_psT", bufs=4, space="PSUM") as pb_psT,
        tc.tile_pool(name="pb_psM", bufs=4, space="PSUM") as pb_psM,
        tc.tile_pool(name="pb_stage", bufs=2) as pb_stage,
    ):
        stages = [None] * RT
        for c in range(NCH):
            pair, side = divmod(c, PAIR)
            if side == 0 and pair >= 2:
                load_pair(pair)
            traw = traw_tiles[pair][:, side * TPC:(side + 1) * TPC]

            nrm = pb_nrm.tile([CT, TPC], F32, tag="nrm")
            for t in range(TPC):
                sq = pb_sq.tile([CT, D], F32)
                nc.scalar.activation(
                    sq[:], traw[:, t], AF.Square, accum_out=nrm[:, t:t + 1])
            nrs = pb_nrm.tile([CT, TPC], F32, tag="nrs")
            # sqrt(||t||^2 * exp(-2*logit_scale)) = ||t|| / s
            nc.scalar.activation(nrs[:], nrm[:], AF.Sqrt, scale=inv_s2)
            rcp = pb_nrm.tile([CT, TPC], F32, tag="rcp")
            nc.vector.reciprocal(rcp[:], nrs[:])       # s / ||t||

            ttT = pb_ttT.tile([128, KD, CHW], F32R)
            for t in range(TPC):
                sc = pb_sc.tile([CT, D], F32)
                nc.vector.tensor_scalar(
                    sc[:], traw[:, t], rcp[:, t:t + 1], None, op0=OP.mult)
                pt = pb_psT.tile([128, KD, CT], F32)
                for kc in range(KD):
                    nc.tensor.transpose(
                        pt[:, kc], sc[:, kc * 128:(kc + 1) * 128],
                        ident[:CT, :CT])
                # f32 -> f32r cast-copy (rounds to tf32) on the Scalar
                # engine to keep the Vector engine free for the mask apply.
                nc.scalar.copy(ttT[:, :, t * CT:(t + 1) * CT], pt[:])

            g, pos = divmod(c, GRP)
            for rt in range(RT):
                pm = pb_psM.tile([128, CHW], F32)
                for kc in range(KD):
                    nc.tensor.matmul(
                        pm[:],
                        imgT_r[:, kc, rt * 128:(rt + 1) * 128],
                        ttT[:, kc],
                        start=(kc == 0), stop=(kc == KD - 1),
                    )
                if pos == 0:
                    stages[rt] = pb_stage.tile(
                        [128, GRP * CHW], F32, tag=f"stg{rt}", name=f"stg{rt}")
                selb = sels[rt][:, c * (CHW // CPT):(c + 1) * (CHW // CPT)]
                selb = selb.broadcast_to([128, CHW // CPT, CPT])
                dst = stages[rt][:, pos * CHW:(pos + 1) * CHW]
                nc.vector.tensor_tensor(
                    dst.rearrange("p (a b) -> p a b", b=CPT),
                    pm[:].rearrange("p (a b) -> p a b", b=CPT),
                    selb, op=OP.mult)
                if pos == GRP - 1:
                    # Stores on the SWDGE queue; loads keep the Sync queue.
                    nc.gpsimd.dma_start(
                        out[rt * 128:(rt + 1) * 128,
                            g * GRP * CHW:(g + 1) * GRP * CHW],
                        stages[rt][:])


def _build(k: int, inv_s2: float):
    nc = bacc.Bacc("TRN2", target_bir_lowering=False, debug=False)
    img = nc.dram_tensor("img", [RLOC, D], F32, kind="ExternalInput").ap()
    proto = nc.dram_tensor("proto", [NP, D], F32, kind="ExternalInput").ap()
    text = nc.dram_tensor("text", [NC, D], F32, kind="ExternalInput").ap()
    out = nc.dram_tensor("out", [RLOC, NC], F32, kind="ExternalOutput").ap()
    with tile.TileContext(nc) as tc:
        with ExitStack() as ctx:
            _emit(ctx, tc, img, proto, text, out, k, inv_s2)
    nc.compile()
    return nc


def kernel(image_features, ima_proto, text_features_raw, logit_scale, num_test):
    global LAST_RESULTS
    img = np.ascontiguousarray(np.asarray(image_features, dtype=np.float32))
    proto = np.ascontiguousarray(np.asarray(ima_proto, dtype=np.float32))
    text = np.ascontiguousarray(np.asarray(text_features_raw, dtype=np.float32))
    assert img.shape == (B, D) and proto.shape == (NP, D) and text.shape == (NC, D)
    s = float(np.asarray(logit_scale))
    k = min(int(np.asarray(num_test)), NP)
    assert 1 <= k <= 16, f"kernel supports k in [1, 16], got {k}"
    inv_s2 = float(np.exp(-2.0 * s))

    nc = _build(k, inv_s2)
    in_maps = [
        {"img": img[i * RLOC:(i + 1) * RLOC], "proto": proto, "text": text}
        for i in range(NCORES)
    ]
    trace = bool(int(os.environ.get("BASS_KERNEL_TRACE", "0")))
    res = run_bass_kernel_spmd(nc, in_maps, list(range(NCORES)), trace=trace)
    LAST_RESULTS = res
    return np.concatenate([r["out"] for r in res.results], axis=0)

